# revision 5
# baseline (speedup 1.0000x reference)
"""DeepLSTM Trainium2 kernel.

Strategy (data-parallel over batch, 8 cores, no collectives):
  - Host: embedding gather, weight re-tiling to bf16 PE layout, batch
    sharding (8 rows/core), final logits GEMM + length-indexed capture.
  - Device, per core (identical SPMD program):
      Phase A: G0 = X @ Wx0 + b0 for all timesteps (big GEMM).
      Phase B: layer-0 LSTM recurrence. Wh0 stays SBUF-resident; each step
               does 64 (LDWEIGHTS+MATMUL) pairs computing transposed gates
               [gate-dim in partitions, batch in free], then the cell math
               on ACT/DVE over wide tiles, pipelined in two hidden-halves
               so elementwise hides under the PE weight-reload wall.
      Phase C: G1 = Y0 @ Wx1 + b1.
      Phase D: layer-1 recurrence -> y1 (fp32) for every step to DRAM.
  - Host: logits = y1[b, len_b-1] @ W_out + b_out.

Masking in the reference is irrelevant to the output: logits only read
h2 at t = len-1, and every value feeding that is an unmasked h_new.

Device DRAM layouts (partition dim outermost so every DMA is a pure slice):
  xt, y0, y1: [128, T_pad, 4, 8]   (p, t, hidden-slice s, batch b)
  g0, g1:     [128, T_pad+2S, 16, 8] (p, t, perm gate-chunk j, b)
"""

import math
import sys

sys.path.insert(0, "/opt/trn_rl_repo")

import numpy as np
import ml_dtypes

HID = 512
NB = 8  # batch rows per core
NCORES = 8

# Gate-chunk permutation. Gate order in W columns is i,j,f,o (each 4 chunks
# of 128). Half-A covers hidden slices {0,1}, half-B {2,3}; within each half
# the order is i,i,f,f,o,o,j,j so sigmoid covers cols 0:48 and tanh 48:64 of
# the 64-wide per-half gate tile.
PERM = [0, 1, 8, 9, 12, 13, 4, 5, 2, 3, 10, 11, 14, 15, 6, 7]

BF16 = ml_dtypes.bfloat16

_PROG_CACHE = {}


def _build_program(T_pad, S):
    import concourse.bass as bass  # noqa: F401
    import concourse.tile as tile
    import concourse.bacc as bacc
    from concourse import mybir
    from concourse.bass import ds
    from contextlib import ExitStack

    fp32 = mybir.dt.float32
    bf16 = mybir.dt.bfloat16
    AF = mybir.ActivationFunctionType
    ET = mybir.EngineType

    n_body = 2 * S
    assert T_pad % n_body == 0
    n_iters = T_pad // n_body

    nc = bacc.Bacc("TRN2", target_bir_lowering=False, debug=False)

    xt = nc.dram_tensor("xt", [128, T_pad, 4, NB], bf16, kind="ExternalInput").ap()
    w0x = nc.dram_tensor("w0x", [128, 8192], bf16, kind="ExternalInput").ap()
    w0h = nc.dram_tensor("w0h", [128, 8192], bf16, kind="ExternalInput").ap()
    w1x = nc.dram_tensor("w1x", [128, 8192], bf16, kind="ExternalInput").ap()
    w1h = nc.dram_tensor("w1h", [128, 8192], bf16, kind="ExternalInput").ap()
    bg0 = nc.dram_tensor("bg0", [128, 16], fp32, kind="ExternalInput").ap()
    bg1 = nc.dram_tensor("bg1", [128, 16], fp32, kind="ExternalInput").ap()
    y1 = nc.dram_tensor("y1", [128, T_pad, 4, NB], fp32, kind="ExternalOutput").ap()

    def gemm_phase(tc, ctx, name, w_dram, b_dram, rhs_dram, g_dram):
        """g[:, t, j, b] = (x W + b) transposed, for all t; + zero pad rows."""
        nc = tc.nc
        wp = ctx.enter_context(tc.tile_pool(name=f"{name}w", bufs=1))
        w_sb = wp.tile([128, 8192], bf16, tag="w")
        nc.sync.dma_start(out=w_sb[:], in_=w_dram)
        b_sb = wp.tile([128, 16], fp32, tag="b")
        nc.sync.dma_start(out=b_sb[:], in_=b_dram)
        xp = ctx.enter_context(tc.tile_pool(name=f"{name}x", bufs=3))
        pp = ctx.enter_context(tc.tile_pool(name=f"{name}p", bufs=3, space="PSUM"))
        op = ctx.enter_context(tc.tile_pool(name=f"{name}o", bufs=4))

        for t0 in range(0, T_pad, 64):
            tcnt = min(64, T_pad - t0)
            ncols = NB * tcnt
            x_sb = xp.tile([128, 64, 4, NB], bf16, tag="x")
            nc.sync.dma_start(out=x_sb[:, :tcnt], in_=rhs_dram[:, t0 : t0 + tcnt])
            for j in range(16):
                ps = pp.tile([128, 512], fp32, tag="ps")
                for k in range(4):
                    nc.tensor.matmul(
                        ps[:, :ncols],
                        w_sb[:, (k * 16 + j) * 128 : (k * 16 + j + 1) * 128],
                        x_sb[:, :tcnt, k, :],
                        start=(k == 0),
                        stop=(k == 3),
                    )
                o_sb = op.tile([128, 512], fp32, tag="o")
                nc.scalar.activation(
                    o_sb[:, :ncols], ps[:, :ncols], AF.Identity, bias=b_sb[:, j : j + 1]
                )
                nc.sync.dma_start(
                    out=g_dram[:, t0 : t0 + tcnt, j, :], in_=o_sb[:, :ncols]
                )
        # Zero the 2S prefetch-overrun pad rows.
        zt = op.tile([128, S, 16, NB], fp32, tag="z")
        nc.vector.memset(zt[:], 0.0)
        for pi in range(2):
            r0 = T_pad + pi * S
            nc.sync.dma_start(out=g_dram[:, r0 : r0 + S], in_=zt[:])

    def recur_phase(tc, ctx, name, g_dram, wh_dram, y_dram, out_bf16):
        """LSTM recurrence. y_dram: [128,T_pad,4,NB] (bf16 for layer0, where
        it also serves as the h state; fp32 for layer1 with separate bf16 h)."""
        nc = tc.nc
        ydt = bf16 if out_bf16 else fp32
        wp = ctx.enter_context(tc.tile_pool(name=f"{name}w", bufs=1))
        wh_sb = wp.tile([128, 8192], bf16, tag="wh")
        nc.sync.dma_start(out=wh_sb[:], in_=wh_dram)

        st = ctx.enter_context(tc.tile_pool(name=f"{name}s", bufs=1))
        ybE = st.tile([128, S, 4, NB], ydt, tag="ybE")
        ybO = st.tile([128, S, 4, NB], ydt, tag="ybO")
        gE = st.tile([128, S, 16, NB], fp32, tag="gE")
        gO = st.tile([128, S, 16, NB], fp32, tag="gO")
        cst = [
            [
                st.tile([128, 16], fp32, tag=f"c{h}{i}", name=f"{name}c{h}{i}")
                for i in (0, 1)
            ]
            for h in (0, 1)
        ]
        hst = None
        if not out_bf16:
            hst = [
                [
                    st.tile([128, 16], bf16, tag=f"h{h}{i}", name=f"{name}h{h}{i}")
                    for i in (0, 1)
                ]
                for h in (0, 1)
            ]

        pp = ctx.enter_context(tc.tile_pool(name=f"{name}p", bufs=2, space="PSUM"))
        ep = ctx.enter_context(tc.tile_pool(name=f"{name}e", bufs=3))

        # Prologue: zero state, load chunk 0.
        for h in (0, 1):
            nc.vector.memset(cst[h][0][:], 0.0)
        if out_bf16:
            nc.vector.memset(ybO[:, S - 1], 0.0)
        else:
            for h in (0, 1):
                nc.vector.memset(hst[h][0][:], 0.0)
        nc.sync.dma_start(out=gE[:], in_=g_dram[:, 0:S])

        def h_src(par, ybuf, ybuf_prev, dt):
            """APs for the 4 K-chunks of h feeding this step's matmuls."""
            if out_bf16:
                if dt == 0:
                    return [ybuf_prev[:, S - 1, s, :] for s in range(4)]
                return [ybuf[:, dt - 1, s, :] for s in range(4)]
            return [
                hst[s // 2][par][:, (s % 2) * 8 : (s % 2) * 8 + 8] for s in range(4)
            ]

        with tc.For_i(
            0,
            n_iters * n_body,
            n_body,
            hint_engines=(ET.PE, ET.DVE, ET.Activation),
            name=f"{name}loop",
        ) as i:
            for hc, (ybuf, ybuf_prev, gbuf) in enumerate(
                [(ybE, ybO, gE), (ybO, ybE, gO)]
            ):
                if hc == 0:
                    # prefetch odd chunk of this iteration
                    nc.sync.dma_start(out=gO[:], in_=g_dram[:, ds(i + S, S)])
                for dt in range(S):
                    step = hc * S + dt
                    par = step % 2
                    hsrc = h_src(par, ybuf, ybuf_prev, dt)
                    psA = pp.tile([128, 64], fp32, tag="psA")
                    psB = pp.tile([128, 64], fp32, tag="psB")
                    # PE blocks ordered so each half's k23 lands as late as
                    # possible after the producing elementwise finishes.
                    # One accumulation group per PSUM bank (zero-region):
                    # start only on the very first MM into the tile, stop on
                    # the last; per-element has_written handles the rest.
                    for ps, jbase, ks in (
                        (psA, 0, (0, 1)),
                        (psB, 8, (0, 1)),
                        (psA, 0, (2, 3)),
                        (psB, 8, (2, 3)),
                    ):
                        for jj in range(8):
                            j = jbase + jj
                            for k in ks:
                                nc.tensor.matmul(
                                    ps[:, jj * 8 : jj * 8 + 8],
                                    wh_sb[
                                        :, (k * 16 + j) * 128 : (k * 16 + j + 1) * 128
                                    ],
                                    hsrc[k],
                                    start=(ks[0] == 0 and jj == 0 and k == 0),
                                    stop=(ks[1] == 3 and jj == 7 and k == 3),
                                )
                    for hi, ps in ((0, psA), (1, psB)):
                        gsl = gbuf[:, dt, hi * 8 : hi * 8 + 8, :]
                        gt = ep.tile([128, 64], fp32, tag=f"gt{hi}")
                        nc.vector.tensor_add(gt[:], ps[:], gsl)
                        sg = ep.tile([128, 48], fp32, tag=f"sg{hi}")
                        nc.scalar.activation(sg[:], gt[:, 0:48], AF.Sigmoid)
                        tj = ep.tile([128, 16], fp32, tag=f"tj{hi}")
                        nc.scalar.activation(tj[:], gt[:, 48:64], AF.Tanh)
                        m1 = ep.tile([128, 16], fp32, tag=f"m1{hi}")
                        nc.vector.tensor_mul(m1[:], sg[:, 0:16], tj[:])
                        m2 = ep.tile([128, 16], fp32, tag=f"m2{hi}")
                        c_prev = cst[hi][par]
                        c_new = cst[hi][1 - par]
                        nc.vector.tensor_mul(m2[:], sg[:, 16:32], c_prev[:])
                        nc.vector.tensor_add(c_new[:], m1[:], m2[:])
                        tch = ep.tile([128, 16], fp32, tag=f"tc{hi}")
                        nc.scalar.activation(tch[:], c_new[:], AF.Tanh)
                        ysl = ybuf[:, dt, 2 * hi : 2 * hi + 2, :]
                        if out_bf16:
                            nc.vector.tensor_mul(ysl, sg[:, 32:48], tch[:])
                        else:
                            h_new = hst[hi][1 - par]
                            nc.vector.tensor_mul(h_new[:], sg[:, 32:48], tch[:])
                            nc.vector.tensor_copy(ysl, h_new[:])
                # chunk writeback
                nc.sync.dma_start(
                    out=y_dram[:, ds(i + hc * S, S)],
                    in_=ybuf[:],
                )
                if hc == 0:
                    # prefetch next iteration's even chunk (pad rows exist)
                    nc.sync.dma_start(out=gE[:], in_=g_dram[:, ds(i + 2 * S, S)])

    with ExitStack() as ctx:
        tc = ctx.enter_context(tile.TileContext(nc))
        dram = ctx.enter_context(tc.tile_pool(name="dram", bufs=1, space="DRAM"))
        g0 = dram.tile([128, T_pad + 2 * S, 16, NB], fp32, tag="g0")
        g1 = dram.tile([128, T_pad + 2 * S, 16, NB], fp32, tag="g1")
        y0 = dram.tile([128, T_pad, 4, NB], bf16, tag="y0")

        with ExitStack() as c1:
            gemm_phase(tc, c1, "ga", w0x, bg0, xt, g0)
        tc.strict_bb_all_engine_barrier()
        with ExitStack() as c2:
            recur_phase(tc, c2, "ra", g0, w0h, y0, out_bf16=True)
        tc.strict_bb_all_engine_barrier()
        with ExitStack() as c3:
            gemm_phase(tc, c3, "gb", w1x, bg1, y0, g1)
        tc.strict_bb_all_engine_barrier()
        with ExitStack() as c4:
            recur_phase(tc, c4, "rb", g1, w1h, y1, out_bf16=False)

    nc.compile()
    return nc


def _prep_w_half(Wp):
    """[512, 2048] f32 -> [128, 4*16*128] bf16 in PE lhsT tile layout."""
    arr = Wp.reshape(4, 128, 16, 128)[:, :, PERM, :]
    return np.ascontiguousarray(
        np.transpose(arr, (1, 0, 2, 3)).reshape(128, 8192)
    ).astype(BF16)


def _prep_b(b):
    """[2048] f32 -> [128, 16] f32, col j = b[PERM[j]*128 : +128]."""
    return np.ascontiguousarray(b.reshape(16, 128)[PERM].T).astype(np.float32)


def build_in_maps(inputs_np, T_pad):
    ids = np.asarray(inputs_np["inputs"]).astype(np.int64)
    emb = np.asarray(inputs_np["emb"], dtype=np.float32)
    W0 = np.asarray(inputs_np["W0"], dtype=np.float32)
    b0 = np.asarray(inputs_np["b0"], dtype=np.float32)
    W1 = np.asarray(inputs_np["W1"], dtype=np.float32)
    b1 = np.asarray(inputs_np["b1"], dtype=np.float32)

    X = emb[ids[:, :T_pad]]  # [B, T_pad, 512] f32
    B = X.shape[0]
    assert B % NB == 0
    ncores = B // NB

    shared = {
        "w0x": _prep_w_half(W0[:512]),
        "w0h": _prep_w_half(W0[512:]),
        "w1x": _prep_w_half(W1[:512]),
        "w1h": _prep_w_half(W1[512:]),
        "bg0": _prep_b(b0),
        "bg1": _prep_b(b1),
    }
    in_maps = []
    for c in range(ncores):
        Xc = X[c * NB : (c + 1) * NB]  # [NB, T_pad, 512]
        # xt[p, t, s, b] = Xc[b, t, s*128+p]
        xtc = np.ascontiguousarray(
            np.transpose(Xc.reshape(NB, T_pad, 4, 128), (3, 1, 2, 0))
        ).astype(BF16)
        in_maps.append({"xt": xtc, **shared})
    return in_maps


def finish_output(results, inputs_np, T_pad):
    lens = np.asarray(inputs_np["input_length"]).astype(np.int64)
    W_out = np.asarray(inputs_np["W_out"], dtype=np.float32)
    b_out = np.asarray(inputs_np["b_out"], dtype=np.float32)
    B = lens.shape[0]
    last = np.empty((B, HID), np.float32)
    for c in range(B // NB):
        y1c = np.asarray(results[c]["y1"])  # [128, T_pad, 4, NB] f32
        for bb in range(NB):
            b = c * NB + bb
            t = int(lens[b]) - 1
            last[b] = y1c[:, t, :, bb].T.reshape(HID)  # hidden = s*128 + p
    return (last @ W_out + b_out).astype(np.float32)


def kernel(**inputs):
    from concourse import bass_utils

    inputs_np = {k: np.asarray(v) for k, v in inputs.items()}
    lens = inputs_np["input_length"].astype(np.int64)
    S = 25
    max_len = int(lens.max())
    T_full = inputs_np["inputs"].shape[1]
    T_pad = max(2 * S, int(math.ceil(max_len / (2 * S))) * 2 * S)
    if T_pad > T_full:
        T_pad = T_full
    if T_pad < max_len or T_pad % (2 * S) != 0:
        assert T_full % (2 * S) == 0, "unsupported T"
        T_pad = T_full

    key = (T_pad, S)
    if key not in _PROG_CACHE:
        _PROG_CACHE[key] = _build_program(T_pad, S)
    nc = _PROG_CACHE[key]

    in_maps = build_in_maps(inputs_np, T_pad)
    res = bass_utils.run_bass_kernel_spmd(
        nc, in_maps, core_ids=list(range(len(in_maps)))
    )
    return finish_output(res.results, inputs_np, T_pad)


# revision 14
# speedup vs baseline: 397.4596x; 397.4596x over previous
"""DeepLSTM Trainium2 kernel.

Strategy (data-parallel over batch, 8 cores, no collectives):
  - Host: embedding gather, weight re-tiling to bf16 PE layout, batch
    sharding (8 rows/core), final logits GEMM + length-indexed capture.
  - Device, per core (identical SPMD program):
      Phase A: G0 = X @ Wx0 + b0 for all timesteps (big GEMM).
      Phase B: layer-0 LSTM recurrence. Wh0 stays SBUF-resident; each step
               does 64 (LDWEIGHTS+MATMUL) pairs computing transposed gates
               [gate-dim in partitions, batch in free], then the cell math
               on ACT/DVE over wide tiles, pipelined in two hidden-halves
               so elementwise hides under the PE weight-reload wall.
      Phase C: G1 = Y0 @ Wx1 + b1.
      Phase D: layer-1 recurrence -> y1 (fp32) for every step to DRAM.
  - Host: logits = y1[b, len_b-1] @ W_out + b_out.

Masking in the reference is irrelevant to the output: logits only read
h2 at t = len-1, and every value feeding that is an unmasked h_new.

Device DRAM layouts (partition dim outermost so every DMA is a pure slice):
  xt, y0, y1: [128, T_pad, 4, 8]   (p, t, hidden-slice s, batch b)
  g0, g1:     [128, T_pad+2S, 16, 8] (p, t, perm gate-chunk j, b)
"""

import math
import sys

sys.path.insert(0, "/opt/trn_rl_repo")

import numpy as np
import ml_dtypes

HID = 512
NB = 8  # batch rows per core
NCORES = 8

# Gate-chunk permutation. Gate order in W columns is i,j,f,o (each 4 chunks
# of 128). Half-A covers hidden slices {0,1}, half-B {2,3}; within each half
# the order is i,i,f,f,o,o,j,j so sigmoid covers cols 0:48 and tanh 48:64 of
# the 64-wide per-half gate tile.
PERM = [0, 1, 8, 9, 12, 13, 4, 5, 2, 3, 10, 11, 14, 15, 6, 7]

BF16 = ml_dtypes.bfloat16

_PROG_CACHE = {}


def _build_program(T_pad, S):
    import concourse.bass as bass  # noqa: F401
    import concourse.tile as tile
    import concourse.bacc as bacc
    from concourse import mybir
    from concourse.bass import ds
    from contextlib import ExitStack

    fp32 = mybir.dt.float32
    bf16 = mybir.dt.bfloat16
    AF = mybir.ActivationFunctionType
    ET = mybir.EngineType

    n_body = 2 * S
    assert T_pad % n_body == 0
    n_iters = T_pad // n_body

    nc = bacc.Bacc("TRN2", target_bir_lowering=False, debug=False)

    xt = nc.dram_tensor("xt", [128, T_pad, 4, NB], bf16, kind="ExternalInput").ap()
    w0x = nc.dram_tensor("w0x", [128, 8192], bf16, kind="ExternalInput").ap()
    w0h = nc.dram_tensor("w0h", [128, 8192], bf16, kind="ExternalInput").ap()
    w1x = nc.dram_tensor("w1x", [128, 8192], bf16, kind="ExternalInput").ap()
    w1h = nc.dram_tensor("w1h", [128, 8192], bf16, kind="ExternalInput").ap()
    bg0 = nc.dram_tensor("bg0", [128, 16], fp32, kind="ExternalInput").ap()
    bg1 = nc.dram_tensor("bg1", [128, 16], fp32, kind="ExternalInput").ap()
    y1 = nc.dram_tensor("y1", [128, T_pad, 4, NB], fp32, kind="ExternalOutput").ap()

    def gemm_phase(tc, ctx, name, w_dram, b_dram, rhs_dram, g_dram):
        """g[:, j, t, b] = (x W + b) transposed, for all t; + zero pad rows.
        G layout [128, 16, T+2S, NB] keeps the per-(j, t-chunk) writeback one
        contiguous 2KB run per partition."""
        nc = tc.nc
        wp = ctx.enter_context(tc.tile_pool(name=f"{name}w", bufs=1))
        w_sb = wp.tile([128, 8192], bf16, tag="w")
        nc.sync.dma_start(out=w_sb[:], in_=w_dram)
        b_sb = wp.tile([128, 16], fp32, tag="b")
        nc.sync.dma_start(out=b_sb[:], in_=b_dram)
        xp = ctx.enter_context(tc.tile_pool(name=f"{name}x", bufs=3))
        pp = ctx.enter_context(tc.tile_pool(name=f"{name}p", bufs=3, space="PSUM"))
        op = ctx.enter_context(tc.tile_pool(name=f"{name}o", bufs=4))

        for t0 in range(0, T_pad, 64):
            tcnt = min(64, T_pad - t0)
            ncols = NB * tcnt
            x_sb = xp.tile([128, 64, 4, NB], bf16, tag="x")
            nc.sync.dma_start(out=x_sb[:, :tcnt], in_=rhs_dram[:, t0 : t0 + tcnt])
            for j in range(16):
                ps = pp.tile([128, 512], fp32, tag="ps")
                for k in range(4):
                    nc.tensor.matmul(
                        ps[:, :ncols],
                        w_sb[:, (k * 16 + j) * 128 : (k * 16 + j + 1) * 128],
                        x_sb[:, :tcnt, k, :],
                        start=(k == 0),
                        stop=(k == 3),
                    )
                o_sb = op.tile([128, 512], fp32, tag="o")
                nc.scalar.activation(
                    o_sb[:, :ncols], ps[:, :ncols], AF.Identity, bias=b_sb[:, j : j + 1]
                )
                nc.sync.dma_start(
                    out=g_dram[:, j, t0 : t0 + tcnt, :], in_=o_sb[:, :ncols]
                )
        # Zero the 2S prefetch-overrun pad rows.
        zt = op.tile([128, 16, S, NB], fp32, tag="z")
        nc.vector.memset(zt[:], 0.0)
        for pi in range(2):
            r0 = T_pad + pi * S
            nc.sync.dma_start(out=g_dram[:, :, r0 : r0 + S, :], in_=zt[:])

    def recur_phase(tc, ctx, name, g_dram, wh_dram, y_dram, out_bf16):
        """LSTM recurrence. y_dram: [128,T_pad,4,NB] (bf16 for layer0, where
        it also serves as the h state; fp32 for layer1 with separate bf16 h)."""
        from concourse.masks import make_identity
        from bass_rust import add_dep_helper

        nc = tc.nc
        ydt = bf16 if out_bf16 else fp32
        wp = ctx.enter_context(tc.tile_pool(name=f"{name}w", bufs=1))
        wh_sb = wp.tile([128, 8192], bf16, tag="wh")
        nc.sync.dma_start(out=wh_sb[:], in_=wh_dram)
        id_sb = wp.tile([128, 128], fp32, tag="ident")
        make_identity(nc, id_sb[:])

        st = ctx.enter_context(tc.tile_pool(name=f"{name}s", bufs=1))
        ybE = st.tile([128, S, 4, NB], ydt, tag="ybE")
        ybO = st.tile([128, S, 4, NB], ydt, tag="ybO")
        gE = st.tile([128, 16, S, NB], fp32, tag="gE")
        gO = st.tile([128, 16, S, NB], fp32, tag="gO")
        cst = [
            [
                st.tile([128, 16], fp32, tag=f"c{h}{i}", name=f"{name}c{h}{i}")
                for i in (0, 1)
            ]
            for h in (0, 1)
        ]
        hst = None
        if not out_bf16:
            hst = [
                [
                    st.tile([128, 16], bf16, tag=f"h{h}{i}", name=f"{name}h{h}{i}")
                    for i in (0, 1)
                ]
                for h in (0, 1)
            ]

        pp = ctx.enter_context(tc.tile_pool(name=f"{name}p", bufs=2, space="PSUM"))
        ep = ctx.enter_context(tc.tile_pool(name=f"{name}e", bufs=3))

        # Prologue: zero state, load chunk 0.
        for h in (0, 1):
            nc.vector.memset(cst[h][0][:], 0.0)
        if out_bf16:
            nc.vector.memset(ybO[:, S - 1], 0.0)
        else:
            for h in (0, 1):
                nc.vector.memset(hst[h][0][:], 0.0)
        nc.sync.dma_start(out=gE[:], in_=g_dram[:, :, 0:S, :])

        def h_src(par, ybuf, ybuf_prev, dt):
            """APs for the 4 K-chunks of h feeding this step's matmuls."""
            if out_bf16:
                if dt == 0:
                    return [ybuf_prev[:, S - 1, s, :] for s in range(4)]
                return [ybuf[:, dt - 1, s, :] for s in range(4)]
            return [
                hst[s // 2][par][:, (s % 2) * 8 : (s % 2) * 8 + 8] for s in range(4)
            ]

        with tc.For_i(
            0,
            n_iters * n_body,
            n_body,
            hint_engines=(ET.PE, ET.DVE, ET.Activation),
            name=f"{name}loop",
        ) as i:
            for hc, (ybuf, ybuf_prev, gbuf) in enumerate(
                [(ybE, ybO, gE), (ybO, ybE, gO)]
            ):
                if hc == 0:
                    # prefetch odd chunk of this iteration
                    nc.sync.dma_start(out=gO[:], in_=g_dram[:, :, ds(i + S, S), :])
                for dt in range(S):
                    step = hc * S + dt
                    par = step % 2
                    hsrc = h_src(par, ybuf, ybuf_prev, dt)
                    psA = pp.tile([128, 64], fp32, tag="psA")
                    psB = pp.tile([128, 64], fp32, tag="psB")
                    # PE blocks: G-injection first (identity stationary, G as
                    # fp32 moving operand — no h dependency), then k01 blocks
                    # (need h half-A), then k23 (need h half-B, latest data).
                    # Explicit same-engine deps pin this order so each half's
                    # PSUM completes early enough to overlap its cell-math
                    # chain with the other half's matmuls.
                    blocks = []
                    for hi, ps in ((0, psA), (1, psB)):
                        binst = []
                        for jj in range(8):
                            binst.append(
                                nc.tensor.matmul(
                                    ps[:, jj * 8 : jj * 8 + 8],
                                    id_sb[:],
                                    gbuf[:, hi * 8 + jj, dt, :],
                                    start=(jj == 0),
                                    stop=False,
                                )
                            )
                        blocks.append(binst)
                    for ks in ((0, 1), (2, 3)):
                        for jbase, ps in ((0, psA), (8, psB)):
                            binst = []
                            for jj in range(8):
                                j = jbase + jj
                                for k in ks:
                                    binst.append(
                                        nc.tensor.matmul(
                                            ps[:, jj * 8 : jj * 8 + 8],
                                            wh_sb[
                                                :,
                                                (k * 16 + j) * 128 : (k * 16 + j + 1)
                                                * 128,
                                            ],
                                            hsrc[k],
                                            start=False,
                                            stop=(ks[1] == 3 and jj == 7 and k == 3),
                                        )
                                    )
                            blocks.append(binst)
                    for prev_b, next_b in zip(blocks, blocks[1:]):
                        add_dep_helper(
                            next_b[0].ins,
                            prev_b[-1].ins,
                            sync=True,
                            reason="pin per-step PE block order",
                        )
                    for hi, ps in ((0, psA), (1, psB)):
                        sg = ep.tile([128, 48], fp32, tag=f"sg{hi}")
                        nc.scalar.activation(sg[:], ps[:, 0:48], AF.Sigmoid)
                        tj = ep.tile([128, 16], fp32, tag=f"tj{hi}")
                        nc.scalar.activation(tj[:], ps[:, 48:64], AF.Tanh)
                        m1 = ep.tile([128, 16], fp32, tag=f"m1{hi}")
                        nc.vector.tensor_mul(m1[:], sg[:, 0:16], tj[:])
                        m2 = ep.tile([128, 16], fp32, tag=f"m2{hi}")
                        c_prev = cst[hi][par]
                        c_new = cst[hi][1 - par]
                        nc.vector.tensor_mul(m2[:], sg[:, 16:32], c_prev[:])
                        nc.vector.tensor_add(c_new[:], m1[:], m2[:])
                        tch = ep.tile([128, 16], fp32, tag=f"tc{hi}")
                        nc.scalar.activation(tch[:], c_new[:], AF.Tanh)
                        ysl = ybuf[:, dt, 2 * hi : 2 * hi + 2, :]
                        if out_bf16:
                            nc.vector.tensor_mul(ysl, sg[:, 32:48], tch[:])
                        else:
                            h_new = hst[hi][1 - par]
                            nc.vector.tensor_mul(h_new[:], sg[:, 32:48], tch[:])
                            nc.vector.tensor_copy(ysl, h_new[:])
                # chunk writeback
                nc.sync.dma_start(
                    out=y_dram[:, ds(i + hc * S, S)],
                    in_=ybuf[:],
                )
                if hc == 0:
                    # prefetch next iteration's even chunk (pad rows exist)
                    nc.sync.dma_start(
                        out=gE[:], in_=g_dram[:, :, ds(i + 2 * S, S), :]
                    )

    with ExitStack() as ctx:
        tc = ctx.enter_context(tile.TileContext(nc))
        dram = ctx.enter_context(tc.tile_pool(name="dram", bufs=1, space="DRAM"))
        g0 = dram.tile([128, 16, T_pad + 2 * S, NB], fp32, tag="g0")
        g1 = dram.tile([128, 16, T_pad + 2 * S, NB], fp32, tag="g1")
        y0 = dram.tile([128, T_pad, 4, NB], bf16, tag="y0")

        with ExitStack() as c1:
            gemm_phase(tc, c1, "ga", w0x, bg0, xt, g0)
        tc.strict_bb_all_engine_barrier()
        with ExitStack() as c2:
            recur_phase(tc, c2, "ra", g0, w0h, y0, out_bf16=True)
        tc.strict_bb_all_engine_barrier()
        with ExitStack() as c3:
            gemm_phase(tc, c3, "gb", w1x, bg1, y0, g1)
        tc.strict_bb_all_engine_barrier()
        with ExitStack() as c4:
            recur_phase(tc, c4, "rb", g1, w1h, y1, out_bf16=False)

    nc.compile()
    return nc


def _prep_w_half(Wp):
    """[512, 2048] f32 -> [128, 4*16*128] bf16 in PE lhsT tile layout."""
    arr = Wp.reshape(4, 128, 16, 128)[:, :, PERM, :]
    return np.ascontiguousarray(
        np.transpose(arr, (1, 0, 2, 3)).reshape(128, 8192)
    ).astype(BF16)


def _prep_b(b):
    """[2048] f32 -> [128, 16] f32, col j = b[PERM[j]*128 : +128]."""
    return np.ascontiguousarray(b.reshape(16, 128)[PERM].T).astype(np.float32)


def build_in_maps(inputs_np, T_pad):
    ids = np.asarray(inputs_np["inputs"]).astype(np.int64)
    emb = np.asarray(inputs_np["emb"], dtype=np.float32)
    W0 = np.asarray(inputs_np["W0"], dtype=np.float32)
    b0 = np.asarray(inputs_np["b0"], dtype=np.float32)
    W1 = np.asarray(inputs_np["W1"], dtype=np.float32)
    b1 = np.asarray(inputs_np["b1"], dtype=np.float32)

    X = emb[ids[:, :T_pad]]  # [B, T_pad, 512] f32
    B = X.shape[0]
    assert B % NB == 0
    ncores = B // NB

    shared = {
        "w0x": _prep_w_half(W0[:512]),
        "w0h": _prep_w_half(W0[512:]),
        "w1x": _prep_w_half(W1[:512]),
        "w1h": _prep_w_half(W1[512:]),
        "bg0": _prep_b(b0),
        "bg1": _prep_b(b1),
    }
    in_maps = []
    for c in range(ncores):
        Xc = X[c * NB : (c + 1) * NB]  # [NB, T_pad, 512]
        # xt[p, t, s, b] = Xc[b, t, s*128+p]
        xtc = np.ascontiguousarray(
            np.transpose(Xc.reshape(NB, T_pad, 4, 128), (3, 1, 2, 0))
        ).astype(BF16)
        in_maps.append({"xt": xtc, **shared})
    return in_maps


def finish_output(results, inputs_np, T_pad):
    lens = np.asarray(inputs_np["input_length"]).astype(np.int64)
    W_out = np.asarray(inputs_np["W_out"], dtype=np.float32)
    b_out = np.asarray(inputs_np["b_out"], dtype=np.float32)
    B = lens.shape[0]
    last = np.empty((B, HID), np.float32)
    for c in range(B // NB):
        y1c = np.asarray(results[c]["y1"])  # [128, T_pad, 4, NB] f32
        for bb in range(NB):
            b = c * NB + bb
            t = int(lens[b]) - 1
            last[b] = y1c[:, t, :, bb].T.reshape(HID)  # hidden = s*128 + p
    return (last @ W_out + b_out).astype(np.float32)


def kernel(**inputs):
    from concourse import bass_utils

    inputs_np = {k: np.asarray(v) for k, v in inputs.items()}
    lens = inputs_np["input_length"].astype(np.int64)
    S = 50
    max_len = int(lens.max())
    T_full = inputs_np["inputs"].shape[1]
    T_pad = max(2 * S, int(math.ceil(max_len / (2 * S))) * 2 * S)
    if T_pad > T_full:
        T_pad = T_full
    if T_pad < max_len or T_pad % (2 * S) != 0:
        assert T_full % (2 * S) == 0, "unsupported T"
        T_pad = T_full

    key = (T_pad, S)
    if key not in _PROG_CACHE:
        _PROG_CACHE[key] = _build_program(T_pad, S)
    nc = _PROG_CACHE[key]

    in_maps = build_in_maps(inputs_np, T_pad)
    res = bass_utils.run_bass_kernel_spmd(
        nc, in_maps, core_ids=list(range(len(in_maps)))
    )
    return finish_output(res.results, inputs_np, T_pad)


# revision 17
# speedup vs baseline: 961.5145x; 2.4192x over previous
"""DeepLSTM Trainium2 kernel.

Strategy (data-parallel over batch, 8 cores, no collectives):
  - Host: embedding gather, weight re-tiling to bf16 PE layout, batch
    sharding (8 rows/core), final logits GEMM + length-indexed capture.
  - Device, per core (identical SPMD program):
      Phase A: G0 = X @ Wx0 + b0 for all timesteps (big GEMM).
      Phase B: layer-0 LSTM recurrence. Wh0 stays SBUF-resident; each step
               does 64 (LDWEIGHTS+MATMUL) pairs computing transposed gates
               [gate-dim in partitions, batch in free], then the cell math
               on ACT/DVE over wide tiles, pipelined in two hidden-halves
               so elementwise hides under the PE weight-reload wall.
      Phase C: G1 = Y0 @ Wx1 + b1.
      Phase D: layer-1 recurrence -> y1 (fp32) for every step to DRAM.
  - Host: logits = y1[b, len_b-1] @ W_out + b_out.

Masking in the reference is irrelevant to the output: logits only read
h2 at t = len-1, and every value feeding that is an unmasked h_new.

Device DRAM layouts (partition dim outermost so every DMA is a pure slice):
  xt, y0, y1: [128, T_pad, 4, 8]   (p, t, hidden-slice s, batch b)
  g0, g1:     [128, T_pad+2S, 16, 8] (p, t, perm gate-chunk j, b)
"""

import math
import sys

sys.path.insert(0, "/opt/trn_rl_repo")

import numpy as np
import ml_dtypes

HID = 512
NB = 8  # batch rows per core
NCORES = 8

# Gate-chunk permutation. Gate order in W columns is i,j,f,o (each 4 chunks
# of 128). Half-A covers hidden slices {0,1}, half-B {2,3}; within each half
# the order is i,i,f,f,o,o,j,j so sigmoid covers cols 0:48 and tanh 48:64 of
# the 64-wide per-half gate tile.
PERM = [0, 1, 8, 9, 12, 13, 4, 5, 2, 3, 10, 11, 14, 15, 6, 7]

BF16 = ml_dtypes.bfloat16

_PROG_CACHE = {}


def _build_program(T_pad, S):
    import concourse.bass as bass  # noqa: F401
    import concourse.tile as tile
    import concourse.bacc as bacc
    from concourse import mybir
    from concourse.bass import ds
    from contextlib import ExitStack

    fp32 = mybir.dt.float32
    bf16 = mybir.dt.bfloat16
    AF = mybir.ActivationFunctionType
    ET = mybir.EngineType

    n_body = 2 * S
    assert T_pad % n_body == 0
    n_iters = T_pad // n_body

    nc = bacc.Bacc("TRN2", target_bir_lowering=False, debug=False)

    xt = nc.dram_tensor("xt", [128, T_pad, 4, NB], bf16, kind="ExternalInput").ap()
    w0x = nc.dram_tensor("w0x", [128, 8192], bf16, kind="ExternalInput").ap()
    w0h = nc.dram_tensor("w0h", [128, 8192], bf16, kind="ExternalInput").ap()
    w1x = nc.dram_tensor("w1x", [128, 8192], bf16, kind="ExternalInput").ap()
    w1h = nc.dram_tensor("w1h", [128, 8192], bf16, kind="ExternalInput").ap()
    bg0 = nc.dram_tensor("bg0", [128, 16], fp32, kind="ExternalInput").ap()
    bg1 = nc.dram_tensor("bg1", [128, 16], fp32, kind="ExternalInput").ap()
    y1 = nc.dram_tensor("y1", [128, T_pad, 4, NB], fp32, kind="ExternalOutput").ap()

    def gemm_phase(tc, ctx, name, w_dram, b_dram, rhs_dram, g_dram):
        """g[:, j, t, b] = (x W + b) transposed, for all t; + zero pad rows.
        G layout [128, 16, T+2S, NB] keeps the per-(j, t-chunk) writeback one
        contiguous 2KB run per partition."""
        nc = tc.nc
        wp = ctx.enter_context(tc.tile_pool(name=f"{name}w", bufs=1))
        w_sb = wp.tile([128, 8192], bf16, tag="w")
        nc.sync.dma_start(out=w_sb[:], in_=w_dram)
        b_sb = wp.tile([128, 16], fp32, tag="b")
        nc.sync.dma_start(out=b_sb[:], in_=b_dram)
        xp = ctx.enter_context(tc.tile_pool(name=f"{name}x", bufs=3))
        pp = ctx.enter_context(tc.tile_pool(name=f"{name}p", bufs=3, space="PSUM"))
        op = ctx.enter_context(tc.tile_pool(name=f"{name}o", bufs=4))

        for t0 in range(0, T_pad, 64):
            tcnt = min(64, T_pad - t0)
            ncols = NB * tcnt
            x_sb = xp.tile([128, 64, 4, NB], bf16, tag="x")
            nc.sync.dma_start(out=x_sb[:, :tcnt], in_=rhs_dram[:, t0 : t0 + tcnt])
            for j in range(16):
                ps = pp.tile([128, 512], fp32, tag="ps")
                for k in range(4):
                    nc.tensor.matmul(
                        ps[:, :ncols],
                        w_sb[:, (k * 16 + j) * 128 : (k * 16 + j + 1) * 128],
                        x_sb[:, :tcnt, k, :],
                        start=(k == 0),
                        stop=(k == 3),
                    )
                o_sb = op.tile([128, 512], fp32, tag="o")
                nc.scalar.activation(
                    o_sb[:, :ncols], ps[:, :ncols], AF.Identity, bias=b_sb[:, j : j + 1]
                )
                nc.sync.dma_start(
                    out=g_dram[:, j, t0 : t0 + tcnt, :], in_=o_sb[:, :ncols]
                )
        # Zero the 2S prefetch-overrun pad rows.
        zt = op.tile([128, 16, S, NB], fp32, tag="z")
        nc.vector.memset(zt[:], 0.0)
        for pi in range(2):
            r0 = T_pad + pi * S
            nc.sync.dma_start(out=g_dram[:, :, r0 : r0 + S, :], in_=zt[:])

    def recur_phase(tc, ctx, name, g_dram, wh_dram, y_dram, out_bf16):
        """LSTM recurrence. y_dram: [128,T_pad,4,NB] (bf16 for layer0, where
        it also serves as the h state; fp32 for layer1 with separate bf16 h)."""
        from bass_rust import add_dep_helper

        nc = tc.nc
        ydt = bf16 if out_bf16 else fp32
        wp = ctx.enter_context(tc.tile_pool(name=f"{name}w", bufs=1))
        wh_sb = wp.tile([128, 8192], bf16, tag="wh")
        nc.sync.dma_start(out=wh_sb[:], in_=wh_dram)

        st = ctx.enter_context(tc.tile_pool(name=f"{name}s", bufs=1))
        ybE = st.tile([128, S, 4, NB], ydt, tag="ybE")
        ybO = st.tile([128, S, 4, NB], ydt, tag="ybO")
        gE = st.tile([128, 16, S, NB], fp32, tag="gE")
        gO = st.tile([128, 16, S, NB], fp32, tag="gO")
        cst = [
            [
                st.tile([128, 16], fp32, tag=f"c{h}{i}", name=f"{name}c{h}{i}")
                for i in (0, 1)
            ]
            for h in (0, 1)
        ]
        hst = None
        if not out_bf16:
            hst = [
                [
                    st.tile([128, 16], bf16, tag=f"h{h}{i}", name=f"{name}h{h}{i}")
                    for i in (0, 1)
                ]
                for h in (0, 1)
            ]

        pp = ctx.enter_context(tc.tile_pool(name=f"{name}p", bufs=2, space="PSUM"))
        ep = ctx.enter_context(tc.tile_pool(name=f"{name}e", bufs=3))

        # Prologue: zero state, load chunk 0.
        for h in (0, 1):
            nc.vector.memset(cst[h][0][:], 0.0)
        if out_bf16:
            nc.vector.memset(ybO[:, S - 1], 0.0)
        else:
            for h in (0, 1):
                nc.vector.memset(hst[h][0][:], 0.0)
        nc.sync.dma_start(out=gE[:], in_=g_dram[:, :, 0:S, :])

        def h_src(par, ybuf, ybuf_prev, dt):
            """APs for the 4 K-chunks of h feeding this step's matmuls."""
            if out_bf16:
                if dt == 0:
                    return [ybuf_prev[:, S - 1, s, :] for s in range(4)]
                return [ybuf[:, dt - 1, s, :] for s in range(4)]
            return [
                hst[s // 2][par][:, (s % 2) * 8 : (s % 2) * 8 + 8] for s in range(4)
            ]

        with tc.For_i(
            0,
            n_iters * n_body,
            n_body,
            hint_engines=(ET.PE, ET.DVE, ET.Activation),
            name=f"{name}loop",
        ) as i:
            for hc, (ybuf, ybuf_prev, gbuf) in enumerate(
                [(ybE, ybO, gE), (ybO, ybE, gO)]
            ):
                if hc == 0:
                    # prefetch odd chunk of this iteration
                    nc.sync.dma_start(out=gO[:], in_=g_dram[:, :, ds(i + S, S), :])
                for dt in range(S):
                    step = hc * S + dt
                    par = step % 2
                    hsrc = h_src(par, ybuf, ybuf_prev, dt)
                    psA = pp.tile([128, 64], fp32, tag="psA")
                    psB = pp.tile([128, 64], fp32, tag="psB")
                    # PE blocks: k01 first (need h half-A only), then k23
                    # (need h half-B, the latest-arriving data). Explicit
                    # same-engine deps pin this order so each half's PSUM
                    # completes early enough to overlap its cell-math chain
                    # with the other half's matmuls.
                    blocks = []
                    for ks in ((0, 1), (2, 3)):
                        for jbase, ps in ((0, psA), (8, psB)):
                            binst = []
                            for jj in range(8):
                                j = jbase + jj
                                for k in ks:
                                    binst.append(
                                        nc.tensor.matmul(
                                            ps[:, jj * 8 : jj * 8 + 8],
                                            wh_sb[
                                                :,
                                                (k * 16 + j) * 128 : (k * 16 + j + 1)
                                                * 128,
                                            ],
                                            hsrc[k],
                                            start=(ks[0] == 0 and jj == 0 and k == 0),
                                            stop=(ks[1] == 3 and jj == 7 and k == 3),
                                        )
                                    )
                            blocks.append(binst)
                    for prev_b, next_b in zip(blocks, blocks[1:]):
                        add_dep_helper(
                            next_b[0].ins,
                            prev_b[-1].ins,
                            sync=True,
                            reason="pin per-step PE block order",
                        )
                    for hi, ps in ((0, psA), (1, psB)):
                        gsl = gbuf[:, hi * 8 : hi * 8 + 8, dt, :]
                        gt = ep.tile([128, 64], fp32, tag=f"gt{hi}")
                        nc.vector.tensor_add(gt[:], ps[:], gsl)
                        sg = ep.tile([128, 48], fp32, tag=f"sg{hi}")
                        nc.scalar.activation(sg[:], gt[:, 0:48], AF.Sigmoid)
                        tj = ep.tile([128, 16], fp32, tag=f"tj{hi}")
                        nc.scalar.activation(tj[:], gt[:, 48:64], AF.Tanh)
                        m1 = ep.tile([128, 16], fp32, tag=f"m1{hi}")
                        nc.vector.tensor_mul(m1[:], sg[:, 0:16], tj[:])
                        m2 = ep.tile([128, 16], fp32, tag=f"m2{hi}")
                        c_prev = cst[hi][par]
                        c_new = cst[hi][1 - par]
                        nc.vector.tensor_mul(m2[:], sg[:, 16:32], c_prev[:])
                        nc.vector.tensor_add(c_new[:], m1[:], m2[:])
                        tch = ep.tile([128, 16], fp32, tag=f"tc{hi}")
                        nc.scalar.activation(tch[:], c_new[:], AF.Tanh)
                        ysl = ybuf[:, dt, 2 * hi : 2 * hi + 2, :]
                        if out_bf16:
                            nc.vector.tensor_mul(ysl, sg[:, 32:48], tch[:])
                        else:
                            h_new = hst[hi][1 - par]
                            nc.vector.tensor_mul(h_new[:], sg[:, 32:48], tch[:])
                            nc.vector.tensor_copy(ysl, h_new[:])
                # chunk writeback
                nc.sync.dma_start(
                    out=y_dram[:, ds(i + hc * S, S)],
                    in_=ybuf[:],
                )
                if hc == 0:
                    # prefetch next iteration's even chunk (pad rows exist)
                    nc.sync.dma_start(
                        out=gE[:], in_=g_dram[:, :, ds(i + 2 * S, S), :]
                    )

    with ExitStack() as ctx:
        tc = ctx.enter_context(tile.TileContext(nc))
        dram = ctx.enter_context(tc.tile_pool(name="dram", bufs=1, space="DRAM"))
        g0 = dram.tile([128, 16, T_pad + 2 * S, NB], fp32, tag="g0")
        g1 = dram.tile([128, 16, T_pad + 2 * S, NB], fp32, tag="g1")
        y0 = dram.tile([128, T_pad, 4, NB], bf16, tag="y0")

        with ExitStack() as c1:
            gemm_phase(tc, c1, "ga", w0x, bg0, xt, g0)
        tc.strict_bb_all_engine_barrier()
        with ExitStack() as c2:
            recur_phase(tc, c2, "ra", g0, w0h, y0, out_bf16=True)
        tc.strict_bb_all_engine_barrier()
        with ExitStack() as c3:
            gemm_phase(tc, c3, "gb", w1x, bg1, y0, g1)
        tc.strict_bb_all_engine_barrier()
        with ExitStack() as c4:
            recur_phase(tc, c4, "rb", g1, w1h, y1, out_bf16=False)

    nc.compile()
    return nc


def _prep_w_half(Wp):
    """[512, 2048] f32 -> [128, 4*16*128] bf16 in PE lhsT tile layout."""
    arr = Wp.reshape(4, 128, 16, 128)[:, :, PERM, :]
    return np.ascontiguousarray(
        np.transpose(arr, (1, 0, 2, 3)).reshape(128, 8192)
    ).astype(BF16)


def _prep_b(b):
    """[2048] f32 -> [128, 16] f32, col j = b[PERM[j]*128 : +128]."""
    return np.ascontiguousarray(b.reshape(16, 128)[PERM].T).astype(np.float32)


def build_in_maps(inputs_np, T_pad):
    ids = np.asarray(inputs_np["inputs"]).astype(np.int64)
    emb = np.asarray(inputs_np["emb"], dtype=np.float32)
    W0 = np.asarray(inputs_np["W0"], dtype=np.float32)
    b0 = np.asarray(inputs_np["b0"], dtype=np.float32)
    W1 = np.asarray(inputs_np["W1"], dtype=np.float32)
    b1 = np.asarray(inputs_np["b1"], dtype=np.float32)

    X = emb[ids[:, :T_pad]]  # [B, T_pad, 512] f32
    B = X.shape[0]
    assert B % NB == 0
    ncores = B // NB

    shared = {
        "w0x": _prep_w_half(W0[:512]),
        "w0h": _prep_w_half(W0[512:]),
        "w1x": _prep_w_half(W1[:512]),
        "w1h": _prep_w_half(W1[512:]),
        "bg0": _prep_b(b0),
        "bg1": _prep_b(b1),
    }
    in_maps = []
    for c in range(ncores):
        Xc = X[c * NB : (c + 1) * NB]  # [NB, T_pad, 512]
        # xt[p, t, s, b] = Xc[b, t, s*128+p]
        xtc = np.ascontiguousarray(
            np.transpose(Xc.reshape(NB, T_pad, 4, 128), (3, 1, 2, 0))
        ).astype(BF16)
        in_maps.append({"xt": xtc, **shared})
    return in_maps


def finish_output(results, inputs_np, T_pad):
    lens = np.asarray(inputs_np["input_length"]).astype(np.int64)
    W_out = np.asarray(inputs_np["W_out"], dtype=np.float32)
    b_out = np.asarray(inputs_np["b_out"], dtype=np.float32)
    B = lens.shape[0]
    last = np.empty((B, HID), np.float32)
    for c in range(B // NB):
        y1c = np.asarray(results[c]["y1"])  # [128, T_pad, 4, NB] f32
        for bb in range(NB):
            b = c * NB + bb
            t = int(lens[b]) - 1
            last[b] = y1c[:, t, :, bb].T.reshape(HID)  # hidden = s*128 + p
    return (last @ W_out + b_out).astype(np.float32)


def kernel(**inputs):
    from concourse import bass_utils

    inputs_np = {k: np.asarray(v) for k, v in inputs.items()}
    lens = inputs_np["input_length"].astype(np.int64)
    S = 50
    max_len = int(lens.max())
    T_full = inputs_np["inputs"].shape[1]
    T_pad = max(2 * S, int(math.ceil(max_len / (2 * S))) * 2 * S)
    if T_pad > T_full:
        T_pad = T_full
    if T_pad < max_len or T_pad % (2 * S) != 0:
        assert T_full % (2 * S) == 0, "unsupported T"
        T_pad = T_full

    key = (T_pad, S)
    if key not in _PROG_CACHE:
        _PROG_CACHE[key] = _build_program(T_pad, S)
    nc = _PROG_CACHE[key]

    in_maps = build_in_maps(inputs_np, T_pad)
    res = bass_utils.run_bass_kernel_spmd(
        nc, in_maps, core_ids=list(range(len(in_maps)))
    )
    return finish_output(res.results, inputs_np, T_pad)
